# revision 1
# baseline (speedup 1.0000x reference)
"""DCNv3-3D Trainium2 Bass kernel.

Full inputs in, full output out. 8 NeuronCores, core k = (n, g) = (k//4, k%4):
data-parallel over batch N, tensor-parallel over the G=4 groups (per the
sharding hint). Each core runs the whole pipeline for its (n, g): in-proj,
depthwise conv + LN + GELU, offset and mask heads, trilinear deformable
sampling (GPSIMD indirect gather + DVE weighted reduce), and a partial output
projection against out_w[:, g-slice]. The host sums the 4 per-group partials
per batch (pure unshard of the sum-sharded output).

Device layouts (l = z*1024 + y*32 + x in [0, 16384)):
  l = (16*lb + s)*128 + t ;  lb = l//2048 (z-block), s = (l//128)%16, t = l%128
  prep/idx tensors : [128 part = 16*lb+s, free (t, p)]
  sample volume    : [128 part = 16*lb+c, free 14440] 10-z-slice slab per lb,
                     double-ring padded coords (22, 38, 38), slab z0 = max(0,2lb-1)
  dwconv/LN/x1     : [128 part = 64*lh+c, free 8192] z-halves of l
Exactness: z-axis sampling exact for |off_z| < 2.5 (slab reach); y/x exact for
any offset. Measured max |off| on the reference distribution = 0.70.
"""
import os
import numpy as np

N, D, H, W, C, G, K = 2, 16, 32, 32, 64, 4, 3
GC, P, L = C // G, K * K * K, D * H * W
Dp, Hp, Wp = 22, 38, 38
SLAB = 10
ROWV = Hp * Wp                    # 1444
VOLSZ = SLAB * ROWV               # 14440
VOL0W = 36864                     # >= 23*1444, 9*4096
IHW = 11596
EPS = 1e-6
TCP = 8                           # prep chunk (t per chunk)
TCG = 4                           # gather chunk (t per chunk)
DLTS = [0, 1, Wp, Wp + 1, ROWV, ROWV + 1, ROWV + Wp, ROWV + Wp + 1]


def _kpax(p, ax):
    return ((p // 9) - 1, ((p // 3) % 3) - 1, (p % 3) - 1)[ax]


def _ap(t, off, dims):
    import concourse.bass as bass
    return bass.AP(t.tensor, t.offset + off, dims)


# ---------------------------------------------------------------- host prep --
def host_prep(inputs, n, g):
    inp = np.asarray(inputs["input"], np.float32)[n]        # (16,32,32,64)
    flat = inp.reshape(L, C)
    input_t = np.ascontiguousarray(flat.T)                  # [64, L]
    pad = np.zeros((C, 18, 34, 34), np.float32)
    pad[:, 1:17, 1:33, 1:33] = inp.transpose(3, 0, 1, 2)
    padflat = pad.reshape(C, 20808)
    ih = np.zeros((128, IHW), np.float32)
    for lh in range(2):
        lo = lh * 9248
        seg = padflat[:, lo:min(20808, lo + IHW)]
        ih[lh * 64:lh * 64 + 64, :seg.shape[1]] = seg
    in_w = np.asarray(inputs["in_w"], np.float32)
    in_w16 = np.ascontiguousarray(in_w[g * GC:(g + 1) * GC, :].T)     # [64,16]
    in_b16 = np.ascontiguousarray(
        np.asarray(inputs["in_b"], np.float32)[g * GC:(g + 1) * GC][:, None])
    off_w = np.asarray(inputs["off_w"], np.float32)
    off_b = np.asarray(inputs["off_b"], np.float32)
    mask_w = np.asarray(inputs["mask_w"], np.float32)
    mask_b = np.asarray(inputs["mask_b"], np.float32)
    W108 = np.zeros((128, 108), np.float32)
    b108 = np.zeros(108, np.float32)
    for ax in range(3):
        for p in range(P):
            W108[0:64, ax * 27 + p] = off_w[g * 81 + p * 3 + ax, :]
            b108[ax * 27 + p] = off_b[g * 81 + p * 3 + ax] + 3.0 + _kpax(p, ax)
    for p in range(P):
        W108[0:64, 81 + p] = mask_w[g * 27 + p, :]
        b108[81 + p] = mask_b[g * 27 + p]
    out_w = np.asarray(inputs["out_w"], np.float32)
    W108[64:128] = W108[0:64]
    ow = out_w[:, g * GC:(g + 1) * GC].T                              # [16,64]
    out_w16 = np.ascontiguousarray(np.tile(ow, (8, 1)))               # [128,64]
    out_b4 = np.ascontiguousarray(
        (np.asarray(inputs["out_b"], np.float32) / 4.0)[:, None])     # [64,1]
    dw_w = np.asarray(inputs["dw_w"], np.float32)
    dwtap = np.zeros((128, 27), np.float32)
    dwb = np.zeros((128, 1), np.float32)
    lng = np.zeros((128, 1), np.float32)
    lnb = np.zeros((128, 1), np.float32)
    for lh in range(2):
        sl = slice(lh * 64, lh * 64 + 64)
        dwtap[sl] = dw_w[:, 0].reshape(C, 27)
        dwb[sl, 0] = np.asarray(inputs["dw_b"], np.float32)
        lng[sl, 0] = np.asarray(inputs["ln_g"], np.float32)
        lnb[sl, 0] = np.asarray(inputs["ln_b"], np.float32)
    cons = np.zeros((128, 4), np.float32)
    for q in range(128):
        lb = q // 16
        cons[q, 0] = q // 8
        cons[q, 1] = max(0, 2 * lb - 1)
        cons[q, 2] = min(max(0, 2 * lb - 1) + 8, 20)
    tt = np.arange(128)
    ixf = np.tile((tt % 32).astype(np.float32)[None, :], (128, 1))
    iyf = ((np.arange(128)[:, None] * 4 + tt[None, :] // 32) % 32).astype(np.float32)
    return dict(input_t=input_t, ih=ih, in_w16=in_w16, in_b16=in_b16,
                W108=W108, b108=b108, out_w16=out_w16, out_b4=out_b4,
                dwtap=dwtap, dwb=dwb, lng=lng, lnb=lnb, cons=cons,
                ixf=np.ascontiguousarray(ixf), iyf=np.ascontiguousarray(iyf))


# ---------------------------------------------------------------- device IR --
def build_nc():
    import concourse.bass as bass
    import concourse.bacc as bacc
    import concourse.mybir as mybir
    import concourse.tile as tile
    global F32, I32, U16, ALU, AF, AXX
    F32 = mybir.dt.float32
    I32 = mybir.dt.int32
    U16 = mybir.dt.int16
    ALU = mybir.AluOpType
    AF = mybir.ActivationFunctionType
    AXX = mybir.AxisListType.X
    nc = bacc.Bacc("TRN2", target_bir_lowering=False)
    d_input_t = nc.dram_tensor("input_t", [64, L], F32, kind="ExternalInput")
    d_ih = nc.dram_tensor("ih", [128, IHW], F32, kind="ExternalInput")
    d_in_w16 = nc.dram_tensor("in_w16", [64, 16], F32, kind="ExternalInput")
    d_in_b16 = nc.dram_tensor("in_b16", [16, 1], F32, kind="ExternalInput")
    d_W108 = nc.dram_tensor("W108", [128, 108], F32, kind="ExternalInput")
    d_b108 = nc.dram_tensor("b108", [108], F32, kind="ExternalInput")
    d_out_w16 = nc.dram_tensor("out_w16", [128, 64], F32, kind="ExternalInput")
    d_out_b4 = nc.dram_tensor("out_b4", [64, 1], F32, kind="ExternalInput")
    d_dwtap = nc.dram_tensor("dwtap", [128, 27], F32, kind="ExternalInput")
    d_dwb = nc.dram_tensor("dwb", [128, 1], F32, kind="ExternalInput")
    d_lng = nc.dram_tensor("lng", [128, 1], F32, kind="ExternalInput")
    d_lnb = nc.dram_tensor("lnb", [128, 1], F32, kind="ExternalInput")
    d_cons = nc.dram_tensor("cons", [128, 4], F32, kind="ExternalInput")
    d_ixf = nc.dram_tensor("ixf", [128, 128], F32, kind="ExternalInput")
    d_iyf = nc.dram_tensor("iyf", [128, 128], F32, kind="ExternalInput")
    d_partial = nc.dram_tensor("partial", [64, L], F32, kind="ExternalOutput")
    d_vol0 = nc.dram_tensor("vol0_hbm", [16, VOL0W], F32, kind="Internal")
    d_uh = nc.dram_tensor("u_hbm", [128, 8 * 3456], F32, kind="Internal")

    with tile.TileContext(nc) as tc:
      with tc.tile_pool(name="const", bufs=1) as const, \
           tc.tile_pool(name="big", bufs=1) as big, \
           tc.tile_pool(name="wk", bufs=1) as wk, \
           tc.tile_pool(name="gw", bufs=2) as gw, \
           tc.tile_pool(name="gws", bufs=1) as gws:

        # ---- constants
        sb_inw16 = const.tile([64, 16], F32)
        nc.sync.dma_start(sb_inw16, d_in_w16[:])
        sb_inb16 = const.tile([16, 1], F32)
        nc.sync.dma_start(sb_inb16, d_in_b16[:])
        sb_W108 = const.tile([128, 108], F32)
        nc.sync.dma_start(sb_W108, d_W108[:])
        sb_outw16 = const.tile([128, 64], F32)
        nc.sync.dma_start(sb_outw16, d_out_w16[:])
        sb_outb4 = const.tile([64, 1], F32)
        nc.sync.dma_start(sb_outb4, d_out_b4[:])
        sb_dwtap = const.tile([128, 27], F32)
        nc.sync.dma_start(sb_dwtap, d_dwtap[:])
        sb_dwb = const.tile([128, 1], F32)
        nc.sync.dma_start(sb_dwb, d_dwb[:])
        sb_lng = const.tile([128, 1], F32)
        nc.sync.dma_start(sb_lng, d_lng[:])
        sb_lnb = const.tile([128, 1], F32)
        nc.sync.dma_start(sb_lnb, d_lnb[:])
        sb_cons = const.tile([128, 4], F32)
        nc.sync.dma_start(sb_cons, d_cons[:])
        sb_b108 = const.tile([128, 108], F32)
        nc.sync.dma_start(sb_b108, bass.AP(d_b108, 0, [[0, 128], [1, 108]]))
        sb_ones = const.tile([128, 128], F32)
        nc.vector.memset(sb_ones, 1.0)
        sb_eps = const.tile([128, 1], F32)
        nc.vector.memset(sb_eps, EPS)

        sb_ixf = const.tile([128, 128], F32)
        nc.sync.dma_start(sb_ixf, d_ixf[:])
        sb_iyf = const.tile([128, 128], F32)
        nc.sync.dma_start(sb_iyf, d_iyf[:])

        # ---- persistent big tiles
        sb_ih = big.tile([128, IHW], F32, tag="ihvol")      # later: vol slab
        nc.sync.dma_start(sb_ih, d_ih[:])
        sb_x1 = big.tile([128, 8192], F32, tag="x1")        # later: gather acc
        sb_idx = big.tile([128, 128, 27], U16, tag="idx")
        sb_res = big.tile([128, 128, 16], F32, tag="res")

        # ---- P1: x16 = in-proj, scattered into HBM vol0 (zeroed first)
        with tc.tile_pool(name="io1", bufs=2) as io1, \
             tc.tile_pool(name="ps1", bufs=2, space="PSUM") as psum1:

            for ch in range(32):
                ibuf = io1.tile([64, 512], F32, tag="ibuf")
                nc.sync.dma_start(ibuf, d_input_t[:, ch * 512:(ch + 1) * 512])
                ps = psum1.tile([16, 512], F32, tag="ps16")
                nc.tensor.matmul(ps, sb_inw16, ibuf, start=True, stop=True)
                xb = io1.tile([16, 512], F32, tag="xb")
                nc.scalar.activation(xb, ps, AF.Identity, bias=sb_inb16,
                                     scale=1.0)
                z, yh = ch // 2, ch % 2
                nc.sync.dma_start(
                    bass.AP(d_vol0, (z + 3) * ROWV + (yh * 16 + 3) * Wp + 3,
                            [[VOL0W, 16], [Wp, 16], [1, 32]]),
                    xb.rearrange("c (y x) -> c y x", y=16))

        # ---- P2: dwconv + LN + GELU -> x1 [128 = 64lh+c, 8192]
        with tc.tile_pool(name="ps2", bufs=2, space="PSUM") as psum2:
            for ch in range(16):
                z, yh = ch // 2, ch % 2
                off0 = (z + 1) * 1156 + (yh * 16 + 1) * 34 + 1
                yc = wk.tile([128, 16, 32], F32, tag="yc")
                for tap in range(27):
                    kz, ky, kx = tap // 9, (tap // 3) % 3, tap % 3
                    dlt = (kz - 1) * 1156 + (ky - 1) * 34 + (kx - 1)
                    src = _ap(sb_ih, off0 + dlt,
                              [[IHW, 128], [34, 16], [1, 32]])
                    if tap == 0:
                        nc.vector.tensor_scalar(yc, src, sb_dwtap[:, 0:1],
                                                sb_dwb, ALU.mult, ALU.add)
                    else:
                        nc.vector.scalar_tensor_tensor(
                            yc, src, sb_dwtap[:, tap:tap + 1], yc,
                            ALU.mult, ALU.add)
                ycf = yc.rearrange("q a b -> q (a b)")
                sq = wk.tile([128, 512], F32, tag="sq")
                nc.scalar.activation(sq, ycf, AF.Square)
                mu = wk.tile([128, 512], F32, tag="mu")
                s2 = wk.tile([128, 512], F32, tag="s2")
                for lh in range(2):
                    sl = slice(lh * 64, lh * 64 + 64)
                    ps1_ = psum2.tile([128, 512], F32, tag="psl")
                    nc.tensor.matmul(ps1_, sb_ones[sl], ycf[sl],
                                     start=True, stop=True)
                    nc.scalar.activation(mu[sl], ps1_[0:64], AF.Identity,
                                         scale=1.0 / 64)
                    ps2_ = psum2.tile([128, 512], F32, tag="psl2")
                    nc.tensor.matmul(ps2_, sb_ones[sl], sq[sl],
                                     start=True, stop=True)
                    nc.scalar.activation(s2[sl], ps2_[0:64], AF.Identity,
                                         scale=1.0 / 64)
                nc.scalar.activation(sq, mu, AF.Square)
                nc.vector.tensor_sub(s2, s2, sq)
                nc.scalar.activation(s2, s2, AF.Sqrt, bias=sb_eps[0:128],
                                     scale=1.0)
                nc.vector.reciprocal(s2, s2)
                nc.vector.tensor_sub(ycf, ycf, mu)
                nc.vector.tensor_mul(ycf, ycf, s2)
                nc.scalar.activation(sb_x1[:, z * 1024 + yh * 512:
                                           z * 1024 + yh * 512 + 512],
                                     ycf, AF.Gelu, bias=sb_lnb, scale=sb_lng)

        # ---- P3: volume slabs (interior-only reads; ring stays zero)
        sb_vol = big.tile([128, VOLSZ], F32, tag="ihvol")
        nc.vector.memset(sb_vol, 0.0)
        for lb in range(8):
            zb = max(0, 2 * lb - 1)
            for zz in range(max(zb, 3), min(zb + 10, 19)):
                nc.sync.dma_start(
                    _ap(sb_vol, 16 * lb * VOLSZ + (zz - zb) * ROWV + 3 * Wp + 3,
                        [[VOLSZ, 16], [Wp, 32], [1, 32]]),
                    bass.AP(d_vol0, zz * ROWV + 3 * Wp + 3,
                            [[VOL0W, 16], [Wp, 32], [1, 32]]))

        # ---- P4+P5: heads (PSUM-resident) + prep per t-chunk
        FW = TCP * 27
        with tc.tile_pool(name="ps5", bufs=2, space="PSUM") as psum5:
            for ch in range(128 // TCP):
                psT = psum5.tile([128, TCP, 128], F32, tag="psT")
                for tw in range(TCP):
                    t = ch * TCP + tw
                    for lh in range(2):
                        lhsT = _ap(sb_x1, lh * 64 * 8192 + t,
                                   [[8192, 64], [128, 64]])
                        nc.tensor.matmul(psT[lh * 64:lh * 64 + 64, tw, 0:108],
                                         lhsT, sb_W108[lh * 64:lh * 64 + 64],
                                         start=True, stop=True)
                ts = slice(ch * TCP, (ch + 1) * TCP)
                r3 = lambda a: a.rearrange("q (t p) -> q t p", p=27)
                q_ = wk.tile([128, FW], F32, tag="q")
                ei = wk.tile([128, FW], I32, tag="ei")
                fr, cc = [None] * 3, [None] * 3
                for ax in range(3):
                    Tsl = psT[:, :, ax * 27:(ax + 1) * 27]
                    bb = _ap(sb_b108, ax * 27, [[108, 128], [0, TCP], [1, 27]])
                    nc.vector.tensor_tensor(r3(q_), Tsl, bb, ALU.add)
                    ef = wk.tile([128, FW], F32, tag=f"ef{ax}")
                    nc.vector.tensor_copy(ei, q_)
                    nc.vector.tensor_copy(ef, ei)
                    cmp_ = wk.tile([128, FW], F32, tag="cmp")
                    nc.vector.tensor_tensor(cmp_, ef, q_, ALU.is_gt)
                    nc.vector.tensor_sub(ef, ef, cmp_)
                    f_ = wk.tile([128, FW], F32, tag=f"f{ax}")
                    nc.vector.tensor_sub(f_, q_, ef)
                    fr[ax] = f_
                    if ax == 0:
                        rb = _ap(sb_ixf, ch * TCP,
                                 [[128, 128], [1, TCP], [0, 27]])
                        nc.vector.tensor_tensor(r3(ef), r3(ef), rb, ALU.add)
                        nc.vector.tensor_scalar(ef, ef, 0.0, 36.0,
                                                ALU.max, ALU.min)
                    elif ax == 1:
                        rb = _ap(sb_iyf, ch * TCP,
                                 [[128, 128], [1, TCP], [0, 27]])
                        nc.vector.tensor_tensor(r3(ef), r3(ef), rb, ALU.add)
                        nc.vector.tensor_scalar(ef, ef, 0.0, 36.0,
                                                ALU.max, ALU.min)
                    else:
                        nc.vector.tensor_scalar(ef, ef, sb_cons[:, 0:1],
                                                sb_cons[:, 1:2],
                                                ALU.add, ALU.max)
                        nc.vector.tensor_scalar(ef, ef, sb_cons[:, 2:3],
                                                sb_cons[:, 1:2],
                                                ALU.min, ALU.subtract)
                    cc[ax] = ef
                nc.vector.scalar_tensor_tensor(q_, cc[2], float(Hp), cc[1],
                                               ALU.mult, ALU.add)
                nc.vector.scalar_tensor_tensor(q_, q_, float(Wp), cc[0],
                                               ALU.mult, ALU.add)
                nc.vector.tensor_copy(
                    sb_idx[:, ts, :].rearrange("q t p -> q (t p)"), q_)
                # softmax over p (logits are small: no max subtraction needed)
                me = wk.tile([128, FW], F32, tag="me")
                nc.scalar.activation(r3(me), psT[:, :, 81:108], AF.Exp)
                den = wk.tile([128, TCP], F32, tag="den")
                nc.vector.tensor_reduce(den, r3(me), AXX, ALU.add)
                nc.vector.reciprocal(den, den)
                m_ = wk.tile([128, FW], F32, tag="m")
                db = _ap(den, 0, [[TCP, 128], [1, TCP], [0, 27]])
                nc.vector.tensor_tensor(r3(m_), r3(me), db, ALU.mult)
                # corner weights; pairs written to HBM as they are produced
                a1 = wk.tile([128, FW], F32, tag="a1")
                nc.vector.tensor_mul(a1, m_, fr[2])
                nc.vector.tensor_sub(m_, m_, a1)                # a0
                b01 = wk.tile([128, FW], F32, tag="b01")
                b11 = wk.tile([128, FW], F32, tag="b11")
                nc.vector.tensor_mul(b01, m_, fr[1])
                nc.vector.tensor_sub(m_, m_, b01)               # b00
                nc.vector.tensor_mul(b11, a1, fr[1])
                nc.vector.tensor_sub(a1, a1, b11)               # b10
                for k, byz in enumerate((m_, b01, a1, b11)):
                    up = wk.tile([128, 2, FW], F32, tag="up")
                    nc.vector.tensor_mul(up[:, 1, :], byz, fr[0])
                    nc.vector.tensor_sub(up[:, 0, :], byz, up[:, 1, :])
                    nc.sync.dma_start(
                        bass.AP(d_uh, 2 * k * 3456 + ch * FW,
                                [[8 * 3456, 128], [3456, 2], [1, FW]]),
                        up)

        # ---- P6: gather + weighted reduce
        # urep holds the corner weights replicated across the 16 channel
        # partitions of each lb group, stored s-OUTER: urep[(lb,c), s*TP + tp].
        # The multiply reads it with a strided AP to match the gather order
        # (tp-outer, s-inner).
        JG = TCG * 16 * 27
        TP = TCG * 27
        for ch in range(128 // TCG):
            acc = big.tile([128, JG], F32, tag="x1")        # reuse x1 slot
            tmp = gws.tile([128, JG], F32, tag="tmp")
            idxs = sb_idx[:, ch * TCG:(ch + 1) * TCG, :] \
                .rearrange("q t p -> q (t p)")
            for k in range(8):
                urep = gw.tile([128, JG], F32, tag="urep")
                for lb in range(8):
                    nc.sync.dma_start(
                        _ap(urep, lb * 16 * JG, [[JG, 16], [1, JG]]),
                        bass.AP(d_uh, lb * 16 * 27648 + k * 3456 + ch * TP,
                                [[0, 16], [27648, 16], [1, TP]]))
                gbuf = gw.tile([128, JG], F32, tag="gbuf")
                data = _ap(sb_vol, DLTS[k],
                           [[VOLSZ, 128], [1, VOLSZ - DLTS[k]]])
                nc.gpsimd.ap_gather(gbuf, data, idxs, channels=128,
                                    num_elems=VOLSZ - DLTS[k], d=1,
                                    num_idxs=JG)
                uview = _ap(urep, 0, [[JG, 128], [1, TP], [TP, 16]])
                gview = _ap(gbuf, 0, [[JG, 128], [16, TP], [1, 16]])
                if k == 0:
                    aview = _ap(acc, 0, [[JG, 128], [16, TP], [1, 16]])
                    nc.vector.tensor_tensor(aview, gview, uview, ALU.mult)
                else:
                    tview = _ap(tmp, 0, [[JG, 128], [16, TP], [1, 16]])
                    nc.vector.tensor_tensor(tview, gview, uview, ALU.mult)
                    nc.vector.tensor_add(acc, acc, tmp)
            accv = _ap(acc, 0, [[JG, 128], [16 * 27, TCG], [1, 16], [16, 27]])
            nc.vector.tensor_reduce(sb_res[:, ch * TCG:(ch + 1) * TCG, :],
                                    accv, AXX, ALU.add)

        # ---- P7: partial out-proj -> HBM
        with tc.tile_pool(name="io7", bufs=2) as io7, \
             tc.tile_pool(name="ps7", bufs=2, space="PSUM") as psum7:
            for lb in range(8):
                stage = io7.tile([16, 2048], F32, tag="stage")
                nc.sync.dma_start(
                    stage, _ap(sb_res, lb * 16 * 2048, [[2048, 16], [1, 2048]]))
                for ch in range(4):
                    ps = psum7.tile([64, 512], F32, tag="pso")
                    nc.tensor.matmul(ps, sb_outw16[0:16],
                                     stage[:, ch * 512:(ch + 1) * 512],
                                     start=True, stop=True)
                    ob = io7.tile([64, 512], F32, tag="ob")
                    nc.scalar.activation(ob, ps, AF.Identity, bias=sb_outb4,
                                         scale=1.0)
                    nc.sync.dma_start(
                        d_partial[:, lb * 2048 + ch * 512:
                                  lb * 2048 + (ch + 1) * 512], ob)
    nc.compile()
    return nc


_NC_CACHE = None
_LAST_RESULT = None


def _get_nc():
    global _NC_CACHE
    if _NC_CACHE is None:
        _NC_CACHE = build_nc()
    return _NC_CACHE


def kernel(**inputs):
    from concourse.bass_utils import run_bass_kernel_spmd
    nc = _get_nc()
    keys = ["input_t", "ih", "in_w16", "in_b16", "W108", "b108", "out_w16",
            "out_b4", "dwtap", "dwb", "lng", "lnb", "cons", "ixf", "iyf"]
    in_maps = []
    for k in range(8):
        hp = host_prep(inputs, k // 4, k % 4)
        in_maps.append({key: np.ascontiguousarray(hp[key]) for key in keys})
    global _LAST_RESULT
    res = run_bass_kernel_spmd(nc, in_maps, core_ids=list(range(8)),
                               trace=bool(int(os.environ.get("KTRACE", "0"))))
    _LAST_RESULT = res
    out = np.zeros((N, L, C), np.float32)
    for k in range(8):
        part = res.results[k]["partial"]          # [64, L] cols (lb, t, s)
        a = part.reshape(C, 8, 128, 16).transpose(1, 3, 2, 0)
        out[k // 4] += a.reshape(L, C)
    return out.reshape(N, D, H, W, C).astype(np.float32)



# revision 4
# speedup vs baseline: 12.2038x; 12.2038x over previous
"""DCNv3-3D Trainium2 Bass kernel (transfer-optimized).

Full inputs in, full output out. 8 NeuronCores, core k = (n, g) = (k//4, k%4):
data-parallel over batch N, tensor-parallel over the G=4 groups. The axon
tunnel (~40 MB/s) dominates wall time, so the I/O contract is minimized:

  H2D per core: xin [16, L] bf16  — the core's 16-channel slice of its batch.
                An on-device AllGather (groups [[0..3],[4..7]]) rebuilds the
                full [64, L] input; the padded dwconv layout is then built
                on-device by strided DMAs (nothing else big is uploaded).
  D2H per core: out16 [16, L] bf16 — an on-device ReduceScatter sums the 4
                per-group partials of the output projection, leaving each
                core with its 16 output channels.

The host dispatcher caches one jitted shard_map executable and recycles the
previous call's (fully overwritten) output buffers as the donated output
operands, so warm calls upload ~5 MB and download ~4 MB total.

Device pipeline per core (unchanged from the validated baseline):
in-proj, depthwise conv + LN + GELU, offset/mask heads, trilinear deformable
sampling (GPSIMD indirect gather + DVE weighted reduce), partial out-proj.
Device layouts (l = z*1024 + y*32 + x in [0, 16384)):
  l = (16*lb + s)*128 + t ;  lb = l//2048 (z-block), s = (l//128)%16, t = l%128
  prep/idx tensors : [128 part = 16*lb+s, free (t, p)]
  sample volume    : [128 part = 16*lb+c, free 14440] 10-z-slice slab per lb,
                     double-ring padded coords (22, 38, 38), slab z0 = max(0,2lb-1)
  dwconv/LN/x1     : [128 part = 64*lh+c, free 8192] z-halves of l
Exactness: z-axis sampling exact for |off_z| < 2.5 (slab reach); y/x exact for
any offset. Measured max |off| on the reference distribution = 0.70.
"""
import os
import numpy as np
import ml_dtypes

BF = ml_dtypes.bfloat16
N, D, H, W, C, G, K = 2, 16, 32, 32, 64, 4, 3
GC, P, L = C // G, K * K * K, D * H * W
Dp, Hp, Wp = 22, 38, 38
SLAB = 10
ROWV = Hp * Wp                    # 1444
VOLSZ = SLAB * ROWV               # 14440
VOL0W = 36864                     # >= 23*1444, 9*4096
IHW = 11596
EPS = 1e-6
TCP = 8                           # prep chunk (t per chunk)
TCG = 4                           # gather chunk (t per chunk)
DLTS = [0, 1, Wp, Wp + 1, ROWV, ROWV + 1, ROWV + Wp, ROWV + Wp + 1]
GROUPS = [[0, 1, 2, 3], [4, 5, 6, 7]]


def _ap(t, off, dims):
    import concourse.bass as bass
    return bass.AP(t.tensor, t.offset + off, dims)


# ------------------------------------------------------- static host consts --
def _static_consts():
    cons = np.zeros((128, 4), np.float32)
    for q in range(128):
        lb = q // 16
        cons[q, 0] = q // 8
        cons[q, 1] = max(0, 2 * lb - 1)
        cons[q, 2] = min(max(0, 2 * lb - 1) + 8, 20)
    tt = np.arange(128)
    ixf = np.tile((tt % 32)[None, :], (128, 1)).astype(BF)
    iyf = ((np.arange(128)[:, None] * 4 + tt[None, :] // 32) % 32).astype(BF)
    # kernel-tap offsets per axis (x, y, z), torch (kw, kh, kd) 'ij' order
    pp = np.arange(P)
    kp = np.stack([(pp // 9) - 1, ((pp // 3) % 3) - 1, (pp % 3) - 1], 0)  # [3,27]
    return cons, np.ascontiguousarray(ixf), np.ascontiguousarray(iyf), kp

_CONS, _IXF, _IYF, _KP = _static_consts()


# ----------------------------------------------------------- per-call inputs --
def prep_inputs(inputs):
    """Vectorized host prep -> dict name -> [8, ...] per-core stacked arrays."""
    inp = np.asarray(inputs["input"], np.float32)           # (2,16,32,32,64)
    xt = inp.reshape(N, L, C).transpose(0, 2, 1)            # (2,64,L)
    xin = np.ascontiguousarray(
        xt.reshape(N, G, GC, L).reshape(8, GC, L)).astype(BF)   # core k=(n,g)

    in_w = np.asarray(inputs["in_w"], np.float32)
    in_w16 = in_w.reshape(G, GC, C).transpose(0, 2, 1).astype(BF)    # (4,64,16)
    in_b16 = np.asarray(inputs["in_b"], np.float32).reshape(G, GC, 1)

    off_w = np.asarray(inputs["off_w"], np.float32).reshape(G, P, 3, C)
    mask_w = np.asarray(inputs["mask_w"], np.float32).reshape(G, P, C)
    W108 = np.concatenate([off_w.transpose(0, 3, 2, 1).reshape(G, C, 81),
                           mask_w.transpose(0, 2, 1)], 2)            # (4,64,108)
    off_b = np.asarray(inputs["off_b"], np.float32).reshape(G, P, 3)
    b108 = np.concatenate(
        [off_b.transpose(0, 2, 1).reshape(G, 81) + 3.0 +
         _KP.reshape(81)[None].astype(np.float32),
         np.asarray(inputs["mask_b"], np.float32).reshape(G, P)], 1)  # (4,108)

    out_w = np.asarray(inputs["out_w"], np.float32)
    out_w16 = out_w.reshape(C, G, GC).transpose(1, 2, 0)             # (4,16,64)
    out_b4 = (np.asarray(inputs["out_b"], np.float32) / G)[:, None]  # (64,1)

    dw2 = np.tile(np.asarray(inputs["dw_w"], np.float32)[:, 0]
                  .reshape(C, 27), (2, 1))                           # (128,27)
    t2 = lambda a: np.tile(np.asarray(a, np.float32)[:, None], (2, 1))
    dwb, lng, lnb = t2(inputs["dw_b"]), t2(inputs["ln_g"]), t2(inputs["ln_b"])

    gi = np.tile(np.arange(G), 2)                                    # g per core
    rep8 = lambda a: np.broadcast_to(a[None], (8,) + a.shape)
    return {
        "xin": xin,
        "in_w16": np.ascontiguousarray(in_w16[gi]),
        "in_b16": np.ascontiguousarray(in_b16[gi]),
        "W108": np.ascontiguousarray(W108[gi]),
        "b108": np.ascontiguousarray(b108[gi]),
        "out_w16": np.ascontiguousarray(out_w16[gi]),
        "out_b4": rep8(out_b4),
        "dwtap": rep8(dw2),
        "dwb": rep8(dwb), "lng": rep8(lng), "lnb": rep8(lnb),
        "cons": rep8(_CONS), "ixf": rep8(_IXF), "iyf": rep8(_IYF),
    }


# ---------------------------------------------------------------- device IR --
def build_nc():
    import concourse.bass as bass
    import concourse.bacc as bacc
    import concourse.mybir as mybir
    import concourse.tile as tile
    global F32, I32, U16, BF16, ALU, AF, AXX
    F32 = mybir.dt.float32
    I32 = mybir.dt.int32
    U16 = mybir.dt.int16
    BF16 = mybir.dt.bfloat16
    ALU = mybir.AluOpType
    AF = mybir.ActivationFunctionType
    AXX = mybir.AxisListType.X
    nc = bacc.Bacc("TRN2", target_bir_lowering=False)
    d_xin = nc.dram_tensor("xin", [GC, L], BF16, kind="ExternalInput")
    d_in_w16 = nc.dram_tensor("in_w16", [64, 16], BF16, kind="ExternalInput")
    d_in_b16 = nc.dram_tensor("in_b16", [16, 1], F32, kind="ExternalInput")
    d_W108 = nc.dram_tensor("W108", [64, 108], F32, kind="ExternalInput")
    d_b108 = nc.dram_tensor("b108", [108], F32, kind="ExternalInput")
    d_out_w16 = nc.dram_tensor("out_w16", [16, 64], F32, kind="ExternalInput")
    d_out_b4 = nc.dram_tensor("out_b4", [64, 1], F32, kind="ExternalInput")
    d_dwtap = nc.dram_tensor("dwtap", [128, 27], F32, kind="ExternalInput")
    d_dwb = nc.dram_tensor("dwb", [128, 1], F32, kind="ExternalInput")
    d_lng = nc.dram_tensor("lng", [128, 1], F32, kind="ExternalInput")
    d_lnb = nc.dram_tensor("lnb", [128, 1], F32, kind="ExternalInput")
    d_cons = nc.dram_tensor("cons", [128, 4], F32, kind="ExternalInput")
    d_ixf = nc.dram_tensor("ixf", [128, 128], BF16, kind="ExternalInput")
    d_iyf = nc.dram_tensor("iyf", [128, 128], BF16, kind="ExternalInput")
    d_out16 = nc.dram_tensor("out16", [GC, L], BF16, kind="ExternalOutput")
    d_vol0 = nc.dram_tensor("vol0_hbm", [16, VOL0W], F32, kind="Internal")
    d_uh = nc.dram_tensor("u_hbm", [128, 8 * 3456], F32, kind="Internal")

    with tile.TileContext(nc) as tc:
      with tc.tile_pool(name="dram", bufs=1, space="DRAM") as dram, \
           tc.tile_pool(name="const", bufs=1) as const, \
           tc.tile_pool(name="big", bufs=1) as big, \
           tc.tile_pool(name="wk", bufs=1) as wk, \
           tc.tile_pool(name="gw", bufs=2) as gw, \
           tc.tile_pool(name="gws", bufs=1) as gws:

        # ---- P0: AllGather the 4 channel-slices -> full [64, L] input
        d_xb = dram.tile([GC, L], BF16)
        nc.gpsimd.dma_start(d_xb[:], d_xin[:])
        d_ag = dram.tile([64, L], BF16)
        nc.gpsimd.collective_compute(
            "AllGather", mybir.AluOpType.bypass, replica_groups=GROUPS,
            ins=[d_xb.opt()], outs=[d_ag.opt()])

        # ---- constants
        sb_inw16 = const.tile([64, 16], BF16)
        nc.sync.dma_start(sb_inw16, d_in_w16[:])
        sb_inb16 = const.tile([16, 1], F32)
        nc.sync.dma_start(sb_inb16, d_in_b16[:])
        sb_W108 = const.tile([128, 108], F32)
        nc.sync.dma_start(_ap(sb_W108, 0, [[108, 64], [1, 108]]), d_W108[:])
        nc.sync.dma_start(_ap(sb_W108, 64 * 108, [[108, 64], [1, 108]]),
                          d_W108[:])
        sb_outw16 = const.tile([16, 64], F32)
        nc.sync.dma_start(sb_outw16, d_out_w16[:])
        sb_outb4 = const.tile([64, 1], F32)
        nc.sync.dma_start(sb_outb4, d_out_b4[:])
        sb_dwtap = const.tile([128, 27], F32)
        nc.sync.dma_start(sb_dwtap, d_dwtap[:])
        sb_dwb = const.tile([128, 1], F32)
        nc.sync.dma_start(sb_dwb, d_dwb[:])
        sb_lng = const.tile([128, 1], F32)
        nc.sync.dma_start(sb_lng, d_lng[:])
        sb_lnb = const.tile([128, 1], F32)
        nc.sync.dma_start(sb_lnb, d_lnb[:])
        sb_cons = const.tile([128, 4], F32)
        nc.sync.dma_start(sb_cons, d_cons[:])
        sb_b108 = const.tile([128, 108], F32)
        nc.sync.dma_start(sb_b108, bass.AP(d_b108, 0, [[0, 128], [1, 108]]))
        sb_ones = const.tile([128, 128], F32)
        nc.vector.memset(sb_ones, 1.0)
        sb_eps = const.tile([128, 1], F32)
        nc.vector.memset(sb_eps, EPS)
        sb_ixb = const.tile([128, 128], BF16)
        nc.sync.dma_start(sb_ixb, d_ixf[:])
        sb_iyb = const.tile([128, 128], BF16)
        nc.sync.dma_start(sb_iyb, d_iyf[:])
        sb_ixf = const.tile([128, 128], F32)
        nc.vector.tensor_copy(sb_ixf, sb_ixb)
        sb_iyf = const.tile([128, 128], F32)
        nc.vector.tensor_copy(sb_iyf, sb_iyb)

        # ---- persistent big tiles
        sb_ih = big.tile([128, IHW], F32, tag="ihvol")      # later: vol slab
        sb_x1 = big.tile([128, 8192], F32, tag="x1")        # later: gather acc
        sb_idx = big.tile([128, 128, 27], U16, tag="idx")
        sb_res = big.tile([128, 128, 16], F32, tag="res")

        # ---- P0.5 + P1: build padded dwconv input (bf16 -> f32) and in-proj
        with tc.tile_pool(name="ihb", bufs=1) as ihb, \
             tc.tile_pool(name="ps1", bufs=2, space="PSUM") as psum1, \
             tc.tile_pool(name="io1", bufs=2) as io1:
            sb_ihb = ihb.tile([128, IHW], BF16)
            nc.vector.memset(sb_ihb, 0.0)
            # interior copies: padded pz slice at (pz*1156 + (y+1)*34 + x+1)
            for lh in range(2):
                pzs = range(1, 10) if lh == 0 else range(8, 17)
                for pz in pzs:
                    zin = pz - 1
                    nc.sync.dma_start(
                        _ap(sb_ihb, lh * 64 * IHW + pz * 1156 - lh * 9248 + 35,
                            [[IHW, 64], [34, 32], [1, 32]]),
                        _ap(d_ag, zin * 1024, [[L, 64], [32, 32], [1, 32]]))
            nc.vector.tensor_copy(sb_ih, sb_ihb)

            # in-proj from the gathered input; x16 scattered into HBM vol0
            for ch in range(32):
                ibuf = io1.tile([64, 512], BF16, tag="ibuf")
                nc.sync.dma_start(
                    ibuf, _ap(d_ag, ch * 512, [[L, 64], [1, 512]]))
                ps = psum1.tile([16, 512], F32, tag="ps16")
                nc.tensor.matmul(ps, sb_inw16, ibuf, start=True, stop=True)
                xb = io1.tile([16, 512], F32, tag="xb")
                nc.scalar.activation(xb, ps, AF.Identity, bias=sb_inb16,
                                     scale=1.0)
                z, yh = ch // 2, ch % 2
                nc.sync.dma_start(
                    bass.AP(d_vol0, (z + 3) * ROWV + (yh * 16 + 3) * Wp + 3,
                            [[VOL0W, 16], [Wp, 16], [1, 32]]),
                    xb.rearrange("c (y x) -> c y x", y=16))

        # ---- P2: dwconv + LN + GELU -> x1 [128 = 64lh+c, 8192]
        with tc.tile_pool(name="ps2", bufs=2, space="PSUM") as psum2:
            for ch in range(16):
                z, yh = ch // 2, ch % 2
                off0 = (z + 1) * 1156 + (yh * 16 + 1) * 34 + 1
                yc = wk.tile([128, 16, 32], F32, tag="yc")
                for tap in range(27):
                    kz, ky, kx = tap // 9, (tap // 3) % 3, tap % 3
                    dlt = (kz - 1) * 1156 + (ky - 1) * 34 + (kx - 1)
                    src = _ap(sb_ih, off0 + dlt,
                              [[IHW, 128], [34, 16], [1, 32]])
                    if tap == 0:
                        nc.vector.tensor_scalar(yc, src, sb_dwtap[:, 0:1],
                                                sb_dwb, ALU.mult, ALU.add)
                    else:
                        nc.vector.scalar_tensor_tensor(
                            yc, src, sb_dwtap[:, tap:tap + 1], yc,
                            ALU.mult, ALU.add)
                ycf = yc.rearrange("q a b -> q (a b)")
                sq = wk.tile([128, 512], F32, tag="sq")
                nc.scalar.activation(sq, ycf, AF.Square)
                mu = wk.tile([128, 512], F32, tag="mu")
                s2 = wk.tile([128, 512], F32, tag="s2")
                for lh in range(2):
                    sl = slice(lh * 64, lh * 64 + 64)
                    ps1_ = psum2.tile([128, 512], F32, tag="psl")
                    nc.tensor.matmul(ps1_, sb_ones[sl], ycf[sl],
                                     start=True, stop=True)
                    nc.scalar.activation(mu[sl], ps1_[0:64], AF.Identity,
                                         scale=1.0 / 64)
                    ps2_ = psum2.tile([128, 512], F32, tag="psl2")
                    nc.tensor.matmul(ps2_, sb_ones[sl], sq[sl],
                                     start=True, stop=True)
                    nc.scalar.activation(s2[sl], ps2_[0:64], AF.Identity,
                                         scale=1.0 / 64)
                nc.scalar.activation(sq, mu, AF.Square)
                nc.vector.tensor_sub(s2, s2, sq)
                nc.scalar.activation(s2, s2, AF.Sqrt, bias=sb_eps[0:128],
                                     scale=1.0)
                nc.vector.reciprocal(s2, s2)
                nc.vector.tensor_sub(ycf, ycf, mu)
                nc.vector.tensor_mul(ycf, ycf, s2)
                nc.scalar.activation(sb_x1[:, z * 1024 + yh * 512:
                                           z * 1024 + yh * 512 + 512],
                                     ycf, AF.Gelu, bias=sb_lnb, scale=sb_lng)

        # ---- P3: volume slabs (interior-only reads; ring stays zero)
        sb_vol = big.tile([128, VOLSZ], F32, tag="ihvol")
        nc.vector.memset(sb_vol, 0.0)
        for lb in range(8):
            zb = max(0, 2 * lb - 1)
            for zz in range(max(zb, 3), min(zb + 10, 19)):
                nc.sync.dma_start(
                    _ap(sb_vol, 16 * lb * VOLSZ + (zz - zb) * ROWV + 3 * Wp + 3,
                        [[VOLSZ, 16], [Wp, 32], [1, 32]]),
                    bass.AP(d_vol0, zz * ROWV + 3 * Wp + 3,
                            [[VOL0W, 16], [Wp, 32], [1, 32]]))

        # ---- P4+P5: heads (PSUM-resident) + prep per t-chunk
        FW = TCP * 27
        with tc.tile_pool(name="ps5", bufs=2, space="PSUM") as psum5:
            for ch in range(128 // TCP):
                psT = psum5.tile([128, TCP, 128], F32, tag="psT")
                for tw in range(TCP):
                    t = ch * TCP + tw
                    for lh in range(2):
                        lhsT = _ap(sb_x1, lh * 64 * 8192 + t,
                                   [[8192, 64], [128, 64]])
                        nc.tensor.matmul(psT[lh * 64:lh * 64 + 64, tw, 0:108],
                                         lhsT, sb_W108[lh * 64:lh * 64 + 64],
                                         start=True, stop=True)
                ts = slice(ch * TCP, (ch + 1) * TCP)
                r3 = lambda a: a.rearrange("q (t p) -> q t p", p=27)
                q_ = wk.tile([128, FW], F32, tag="q")
                ei = wk.tile([128, FW], I32, tag="ei")
                fr, cc = [None] * 3, [None] * 3
                for ax in range(3):
                    Tsl = psT[:, :, ax * 27:(ax + 1) * 27]
                    bb = _ap(sb_b108, ax * 27, [[108, 128], [0, TCP], [1, 27]])
                    nc.vector.tensor_tensor(r3(q_), Tsl, bb, ALU.add)
                    ef = wk.tile([128, FW], F32, tag=f"ef{ax}")
                    nc.vector.tensor_copy(ei, q_)
                    nc.vector.tensor_copy(ef, ei)
                    cmp_ = wk.tile([128, FW], F32, tag="cmp")
                    nc.vector.tensor_tensor(cmp_, ef, q_, ALU.is_gt)
                    nc.vector.tensor_sub(ef, ef, cmp_)
                    f_ = wk.tile([128, FW], F32, tag=f"f{ax}")
                    nc.vector.tensor_sub(f_, q_, ef)
                    fr[ax] = f_
                    if ax == 0:
                        rb = _ap(sb_ixf, ch * TCP,
                                 [[128, 128], [1, TCP], [0, 27]])
                        nc.vector.tensor_tensor(r3(ef), r3(ef), rb, ALU.add)
                        nc.vector.tensor_scalar(ef, ef, 0.0, 36.0,
                                                ALU.max, ALU.min)
                    elif ax == 1:
                        rb = _ap(sb_iyf, ch * TCP,
                                 [[128, 128], [1, TCP], [0, 27]])
                        nc.vector.tensor_tensor(r3(ef), r3(ef), rb, ALU.add)
                        nc.vector.tensor_scalar(ef, ef, 0.0, 36.0,
                                                ALU.max, ALU.min)
                    else:
                        nc.vector.tensor_scalar(ef, ef, sb_cons[:, 0:1],
                                                sb_cons[:, 1:2],
                                                ALU.add, ALU.max)
                        nc.vector.tensor_scalar(ef, ef, sb_cons[:, 2:3],
                                                sb_cons[:, 1:2],
                                                ALU.min, ALU.subtract)
                    cc[ax] = ef
                nc.vector.scalar_tensor_tensor(q_, cc[2], float(Hp), cc[1],
                                               ALU.mult, ALU.add)
                nc.vector.scalar_tensor_tensor(q_, q_, float(Wp), cc[0],
                                               ALU.mult, ALU.add)
                nc.vector.tensor_copy(
                    sb_idx[:, ts, :].rearrange("q t p -> q (t p)"), q_)
                # softmax over p (logits are small: no max subtraction needed)
                me = wk.tile([128, FW], F32, tag="me")
                nc.scalar.activation(r3(me), psT[:, :, 81:108], AF.Exp)
                den = wk.tile([128, TCP], F32, tag="den")
                nc.vector.tensor_reduce(den, r3(me), AXX, ALU.add)
                nc.vector.reciprocal(den, den)
                m_ = wk.tile([128, FW], F32, tag="m")
                db = _ap(den, 0, [[TCP, 128], [1, TCP], [0, 27]])
                nc.vector.tensor_tensor(r3(m_), r3(me), db, ALU.mult)
                # corner weights; pairs written to HBM as they are produced
                a1 = wk.tile([128, FW], F32, tag="a1")
                nc.vector.tensor_mul(a1, m_, fr[2])
                nc.vector.tensor_sub(m_, m_, a1)                # a0
                b01 = wk.tile([128, FW], F32, tag="b01")
                b11 = wk.tile([128, FW], F32, tag="b11")
                nc.vector.tensor_mul(b01, m_, fr[1])
                nc.vector.tensor_sub(m_, m_, b01)               # b00
                nc.vector.tensor_mul(b11, a1, fr[1])
                nc.vector.tensor_sub(a1, a1, b11)               # b10
                for k, byz in enumerate((m_, b01, a1, b11)):
                    up = wk.tile([128, 2, FW], F32, tag="up")
                    nc.vector.tensor_mul(up[:, 1, :], byz, fr[0])
                    nc.vector.tensor_sub(up[:, 0, :], byz, up[:, 1, :])
                    nc.sync.dma_start(
                        bass.AP(d_uh, 2 * k * 3456 + ch * FW,
                                [[8 * 3456, 128], [3456, 2], [1, FW]]),
                        up)

        # ---- P6: gather + weighted reduce
        # urep holds the corner weights replicated across the 16 channel
        # partitions of each lb group, stored s-OUTER: urep[(lb,c), s*TP + tp].
        # The multiply reads it with a strided AP to match the gather order
        # (tp-outer, s-inner).
        JG = TCG * 16 * 27
        TP = TCG * 27
        for ch in range(128 // TCG):
            acc = big.tile([128, JG], F32, tag="x1")        # reuse x1 slot
            tmp = gws.tile([128, JG], F32, tag="tmp")
            idxs = sb_idx[:, ch * TCG:(ch + 1) * TCG, :] \
                .rearrange("q t p -> q (t p)")
            for k in range(8):
                urep = gw.tile([128, JG], F32, tag="urep")
                for lb in range(8):
                    nc.sync.dma_start(
                        _ap(urep, lb * 16 * JG, [[JG, 16], [1, JG]]),
                        bass.AP(d_uh, lb * 16 * 27648 + k * 3456 + ch * TP,
                                [[0, 16], [27648, 16], [1, TP]]))
                gbuf = gw.tile([128, JG], F32, tag="gbuf")
                data = _ap(sb_vol, DLTS[k],
                           [[VOLSZ, 128], [1, VOLSZ - DLTS[k]]])
                nc.gpsimd.ap_gather(gbuf, data, idxs, channels=128,
                                    num_elems=VOLSZ - DLTS[k], d=1,
                                    num_idxs=JG)
                uview = _ap(urep, 0, [[JG, 128], [1, TP], [TP, 16]])
                gview = _ap(gbuf, 0, [[JG, 128], [16, TP], [1, 16]])
                if k == 0:
                    aview = _ap(acc, 0, [[JG, 128], [16, TP], [1, 16]])
                    nc.vector.tensor_tensor(aview, gview, uview, ALU.mult)
                else:
                    tview = _ap(tmp, 0, [[JG, 128], [16, TP], [1, 16]])
                    nc.vector.tensor_tensor(tview, gview, uview, ALU.mult)
                    nc.vector.tensor_add(acc, acc, tmp)
            accv = _ap(acc, 0, [[JG, 128], [16 * 27, TCG], [1, 16], [16, 27]])
            nc.vector.tensor_reduce(sb_res[:, ch * TCG:(ch + 1) * TCG, :],
                                    accv, AXX, ALU.add)

        # ---- P7: partial out-proj -> HBM bounce, ReduceScatter, bf16 out
        d_part = dram.tile([64, L], F32)
        with tc.tile_pool(name="io7", bufs=2) as io7, \
             tc.tile_pool(name="ps7", bufs=2, space="PSUM") as psum7:
            for lb in range(8):
                stage = io7.tile([16, 2048], F32, tag="stage")
                nc.sync.dma_start(
                    stage, _ap(sb_res, lb * 16 * 2048, [[2048, 16], [1, 2048]]))
                for ch in range(4):
                    ps = psum7.tile([64, 512], F32, tag="pso")
                    nc.tensor.matmul(ps, sb_outw16,
                                     stage[:, ch * 512:(ch + 1) * 512],
                                     start=True, stop=True)
                    ob = io7.tile([64, 512], F32, tag="ob")
                    nc.scalar.activation(ob, ps, AF.Identity, bias=sb_outb4,
                                         scale=1.0)
                    nc.sync.dma_start(
                        _ap(d_part, lb * 2048 + ch * 512, [[L, 64], [1, 512]]),
                        ob)

        d_rs = dram.tile([GC, L], F32)
        nc.gpsimd.collective_compute(
            "ReduceScatter", mybir.AluOpType.add, replica_groups=GROUPS,
            ins=[d_part.opt()], outs=[d_rs.opt()])
        with tc.tile_pool(name="fin", bufs=1) as fin:
            # spread [16, L] over all 128 partitions as (c, seg) x 2048 cols
            rsb = fin.tile([128, 2048], F32)
            nc.sync.dma_start(
                _ap(rsb, 0, [[2048, 128], [1, 2048]]),
                _ap(d_rs, 0, [[L, 16], [2048, 8], [1, 2048]]))
            o16 = fin.tile([128, 2048], BF16)
            nc.vector.tensor_copy(o16, rsb)
            nc.sync.dma_start(
                bass.AP(d_out16, 0, [[L, 16], [2048, 8], [1, 2048]]),
                _ap(o16, 0, [[2048, 128], [1, 2048]]))
    nc.compile()
    return nc


# ------------------------------------------------------- cached dispatcher --
class _Dispatch:
    """run_bass_via_pjrt, but: jit built once, donated output buffers
    recycled from the previous call (the kernel fully overwrites them)."""

    def __init__(self):
        import jax
        import concourse.mybir as mybir
        from concourse.bass2jax import (install_neuronx_cc_hook,
                                        _bass_exec_p, partition_id_tensor)
        from jax.sharding import Mesh, PartitionSpec
        from jax.experimental.shard_map import shard_map
        install_neuronx_cc_hook()
        self.jax = jax
        nc = build_nc()
        pname = nc.partition_id_tensor.name if nc.partition_id_tensor else None
        in_names, out_names, out_avals = [], [], []
        for alloc in nc.m.functions[0].allocations:
            if not isinstance(alloc, mybir.MemoryLocationSet):
                continue
            name = alloc.memorylocations[0].name
            if alloc.kind == "ExternalInput":
                if name != pname:
                    in_names.append(name)
            elif alloc.kind == "ExternalOutput":
                out_names.append(name)
                out_avals.append(jax.core.ShapedArray(
                    tuple(alloc.tensor_shape), mybir.dt.np(alloc.dtype)))
        self.in_names, self.out_names, self.out_avals = \
            in_names, out_names, out_avals
        n_params, n_outs = len(in_names), len(out_avals)
        all_names = in_names + out_names + ([pname] if pname else [])

        def _body(*args):
            operands = list(args)
            if pname is not None:
                operands.append(partition_id_tensor())
            return tuple(_bass_exec_p.bind(
                *operands, out_avals=tuple(out_avals),
                in_names=tuple(all_names), out_names=tuple(out_names),
                lowering_input_output_aliases=(), sim_require_finite=True,
                sim_require_nnan=True, nc=nc))

        devices = jax.devices()[:8]
        mesh = Mesh(np.asarray(devices), ("core",))
        specs = (PartitionSpec("core"),) * (n_params + n_outs)
        self.sharded = jax.jit(
            shard_map(_body, mesh=mesh, in_specs=specs,
                      out_specs=(PartitionSpec("core"),) * n_outs,
                      check_rep=False),
            donate_argnums=tuple(range(n_params, n_params + n_outs)),
            keep_unused=True)
        self.recycle = None

    def __call__(self, stacked):
        concat_in = [np.ascontiguousarray(stacked[n]).reshape(
            -1, *stacked[n].shape[2:]) for n in self.in_names]
        if self.recycle is None:
            outs_op = [np.zeros((8 * a.shape[0], *a.shape[1:]), a.dtype)
                       for a in self.out_avals]
        else:
            outs_op = self.recycle
        out_arrs = self.sharded(*concat_in, *outs_op)
        outs_np = {n: np.asarray(a).reshape(8, *self.out_avals[i].shape)
                   for i, (n, a) in enumerate(zip(self.out_names, out_arrs))}
        self.recycle = list(out_arrs)
        return outs_np


_DISPATCH = None


def kernel(**inputs):
    global _DISPATCH
    if _DISPATCH is None:
        _DISPATCH = _Dispatch()
    stacked = prep_inputs(inputs)
    if int(os.environ.get("KPROF", "0")):
        return _kernel_traced(stacked)
    res = _DISPATCH(stacked)
    o = res["out16"].astype(np.float32)              # [8, 16, L] cols (lb,t,s)
    out = np.zeros((N, L, C), np.float32)
    for k in range(8):
        n, g = k // 4, k % 4
        a = o[k].reshape(GC, 8, 128, 16).transpose(1, 3, 2, 0)
        out[n, :, g * GC:(g + 1) * GC] = a.reshape(L, GC)
    return out.reshape(N, D, H, W, C)


def _kernel_traced(stacked):
    """Profiling path: one-shot run via run_bass_kernel_spmd(trace=True)."""
    from concourse.bass_utils import run_bass_kernel_spmd
    nc = build_nc()
    in_maps = [{n: np.ascontiguousarray(stacked[n][k])
                for n in stacked} for k in range(8)]
    res = run_bass_kernel_spmd(nc, in_maps, core_ids=list(range(8)),
                               trace=True)
    globals()["_LAST_RESULT"] = res
    out = np.zeros((N, L, C), np.float32)
    for k in range(8):
        n, g = k // 4, k % 4
        a = res.results[k]["out16"].astype(np.float32) \
            .reshape(GC, 8, 128, 16).transpose(1, 3, 2, 0)
        out[n, :, g * GC:(g + 1) * GC] = a.reshape(L, GC)
    return out.reshape(N, D, H, W, C)
